# revision 46
# baseline (speedup 1.0000x reference)
"""LIF spiking-neuron recurrence kernel for Trainium2 (Bass/Tile, 8-core SPMD).

Problem: x [32, 128, 32, 32, 8] f32, time on the LAST axis (T=8).
    u_0 = x_0;  o_t = (u_t > Vth);  u_{t+1} = TAU * u_t * (1 - o_t) + x_{t+1}
Output: spikes o [32, 128, 32, 32, 8] f32 (0.0 / 1.0).

Sharding: pure data-parallel over the batch dim (32 -> 4 per core, 8 cores),
no communication. The host lays each core's shard out t-plane-major
([pixels, T] -> [T, pixels] per 1024-pixel row group) so every on-chip
operand is dense unit-stride. Spikes are exactly 0/1 so the output is
stored as int8, cutting store traffic 4x (per-core HBM traffic 21 MB,
~59 us floor at the ~358 GB/s per-core peak).

With cheap stores, the binding constraint is the Vector engine: fp32
tensor_tensor runs at 1 elem/cycle/lane, and the recurrence nominally needs
three 2-tensor ops per timestep (mask, masked-mult, add) = ~5.8 us per
[128, 2048] plane. This kernel reduces DVE to TWO ops per step by computing
the spike gate on the otherwise-idle Activation engine as a Relu ramp:

    z   = Relu(-BIG*u + BIG*u*)        ACT; u* = nextafter(Vth) so that
                                       z > 0  <=>  u <= Vth (exact for every
                                       f32 input; z >= ~12 whenever nonzero)
    w   = min(TAU*u, z)                DVE scalar_tensor_tensor (mult, min)
                                       == TAU*u*(u <= Vth) exactly, because
                                       TAU*u <= 0.0500000008 < 12 when gated
    o_t = Relu(1 - z) -> int8          ACT; z==0 -> 1, z>=12 -> 0
    u'  = w + x_{t+1}                  DVE tensor_tensor add

TAU*u rounds identically to the reference's TAU*u*(1-o) path, so spike
outputs are bit-exact (including u == Vth exactly, handled by u*).

Engine budget per core (measured): DVE 14 stt + 14 tt + final compares
~= 67 us busy; ACT 28 activations + 14 store issues ~= 68 us; DMA ~54 us.
Stores issue from ACT's HWDGE queue right behind the o they store (SWDGE
via GPSIMD measured ~1 us slower end-to-end); loads issue from SP and
prefetch three steps ahead so DMA pressure stays even.

Rejected alternatives (all measured slower):
  - GPSIMD elementwise (tensor_tensor ~3.4 cyc/elem, tensor_scalar
    ~32 us/plane, and it starves DVE via the shared SBUF port)
  - SWDGE accumulate-DMA for the add (~155 GB/s effective and ~2.5-3.5 us
    latency lands on the recurrence critical path)
  - PE identity-matmul adds (fp32 moving operand decomposes to bf16 pairs
    -> not exact), fp16/bf16 state (too many threshold flips for 2e-2)
"""

import numpy as np

import bass_rust
import concourse.bass as bass
import concourse.mybir as mybir
import concourse.tile as tile
from concourse.bass_utils import run_bass_kernel_spmd

VTH = 0.2
TAU = 0.25

# Gate constants: fma(-BIG, u, B_GATE) with B_GATE = float32(BIG * nextafter
# (float32(0.2))). The result is > 0 exactly when u <= float32(0.2) and the
# smallest positive value it takes is ~12 (one f32 ulp at 0.2 scaled by BIG),
# safely above max(TAU*u) = 0.0500000008, so min(TAU*u, z) never picks z when
# the gate is on. Exact under both fused and round-between multiply-add.
BIG = 1.0e9
B_GATE = 200000016.0

N_CORES = 8
FULL_SHAPE = (32, 128, 32, 32, 8)
B_PER_CORE = FULL_SHAPE[0] // N_CORES  # 4
T = FULL_SHAPE[-1]  # 8

ROWS = 256  # per-core partition rows: 4*128*32*32*8 / FREE
FREE = 16384  # free dim per row
C = FREE // T  # 2048 pixels per partition row
N_TILES = ROWS // 128  # 2

_cache: dict = {}


def _split_multi_waits(nc: bass.Bass) -> int:
    """Hoist all-but-one embedded sync waits onto standalone EventSemaphore
    instructions. The walrus build behind bass2jax rejects >1 sync wait per
    instruction ("Too many sync wait commands"); a standalone wait on the
    same engine stream immediately before is semantically identical."""
    n = 0
    for fn in nc.m.functions:
        for block in fn.blocks:
            out = []
            changed = False
            for ins in block.instructions:
                si = ins.sync_info
                waits = list(si.on_wait) if si is not None else []
                if len(waits) > 1:
                    for k, w in enumerate(waits[:-1]):
                        ev = mybir.InstEventSemaphore(
                            name=f"{ins.name}-hw{k}", ins=[], outs=[]
                        )
                        ev.sync_info = bass_rust.SyncInfo(
                            on_wait=[w], on_update=[]
                        )
                        ev.engine = ins.engine
                        nc.inst_map[ev.name] = ev
                        out.append(ev)
                        n += 1
                    si.on_wait = [waits[-1]]
                    changed = True
                out.append(ins)
            if changed:
                block.instructions = out
    return n


def _build_bass() -> bass.Bass:
    f32 = mybir.dt.float32
    i8 = mybir.dt.int8
    Alu = mybir.AluOpType
    Act = mybir.ActivationFunctionType

    nc = bass.Bass(trn_type="TRN2")
    x_d = nc.dram_tensor("x", [ROWS, FREE], f32, kind="ExternalInput")
    y_d = nc.dram_tensor("y", [ROWS, FREE], i8, kind="ExternalOutput")

    # Non-Copy activations take their bias as a per-partition const AP; only
    # 0.0/1.0 are pre-registered, so add the gate bias. Initializing it on
    # the Scalar queue itself (zero, then Copy-with-bias) keeps the init
    # in-order ahead of the first Relu without an all-engine barrier that
    # would hold up the first loads.
    bias_t = nc.alloc_sbuf_tensor("const-bgate", [128, 1], f32)
    nc.scalar.memzero(bias_t.ap())
    nc.scalar.activation(
        bias_t.ap(), bias_t.ap(), mybir.ActivationFunctionType.Copy,
        bias=B_GATE, scale=1.0,
    )
    nc.const_aps.aps[(f32, B_GATE)] = bias_t.ap()

    with tile.TileContext(nc) as tc:
        with (
            tc.tile_pool(name="pin", bufs=12) as pin,
            tc.tile_pool(name="pout", bufs=4) as pout,
            tc.tile_pool(name="pz", bufs=6) as pz,
            tc.tile_pool(name="pu", bufs=5) as pu,
        ):
            row_sl = [slice(i * 128, (i + 1) * 128) for i in range(N_TILES)]

            def load(i, t):
                p = pin.tile([128, C], f32, tag="xp")
                nc.sync.dma_start(p, x_d[row_sl[i], t * C : (t + 1) * C])
                return p

            # four-step prefetch: planes 0-3 of both tiles land first, the
            # rest stream three steps ahead of their consuming add
            PF = 4
            xp = [[None] * T for _ in range(N_TILES)]
            for t in range(PF):
                for i in range(N_TILES):
                    xp[i][t] = load(i, t)

            u = [xp[i][0] for i in range(N_TILES)]
            for t in range(T - 1):
                # Both tiles' gates are emitted FIRST on ACT each step:
                # z_B must be ready ~4.6us after z_A (when DVE finishes
                # tile A and turns to stt_B), and o/store issues queued in
                # between would push it past that deadline.
                zs = []
                for i in range(N_TILES):
                    if t + PF < T:
                        xp[i][t + PF] = load(i, t + PF)
                    # ACT: spike gate z = Relu(BIG*(u* - u))
                    z = pz.tile([128, C], f32, tag="z")
                    nc.scalar.activation(
                        z, u[i], Act.Relu, bias=B_GATE, scale=-BIG
                    )
                    zs.append(z)
                for i in range(N_TILES):
                    z = zs[i]
                    # ACT: o = Relu(1 - z) in {0, 1} -> int8
                    o_t = pout.tile([128, C], i8, tag="o")
                    nc.scalar.activation(
                        o_t, z, Act.Relu, bias=1.0, scale=-1.0
                    )
                    # DVE: w = min(TAU*u, z) == TAU*u*(u <= Vth), written
                    # straight into the next-state tile, then the add runs
                    # in place - no separate w ring, one fewer allocation
                    # wait per step on the DVE queue
                    un = pu.tile([128, C], f32, tag="u")
                    nc.vector.scalar_tensor_tensor(
                        un, u[i], TAU, z, Alu.mult, Alu.min
                    )
                    # DVE: u' = w + x_{t+1}
                    nc.vector.tensor_tensor(un, un, xp[i][t + 1], Alu.add)
                    # store via ACT's HWDGE queue, right behind the o it
                    # stores (no wait); SWDGE (gpsimd) descriptor-gen runs
                    # in SBUF rings that DVE 2-port ops can lock out
                    nc.scalar.dma_start(
                        y_d[row_sl[i], t * C : (t + 1) * C], o_t
                    )
                    u[i] = un

                    if t == T - 2:
                        # This tile's last-step compare runs right behind
                        # its final add, in halves whose stores overlap, so
                        # the kernel tail is half an op. It stays on DVE:
                        # GPSIMD's tensor_scalar measures ~32us/plane (25x
                        # DVE) and starves DVE via the shared SBUF port.
                        HC = C // 2
                        for h in range(2):
                            o_l = pout.tile([128, HC], i8, tag="oh")
                            nc.vector.tensor_scalar(
                                o_l, un[:, h * HC : (h + 1) * HC],
                                VTH, None, Alu.is_gt,
                            )
                            nc.sync.dma_start(
                                y_d[
                                    row_sl[i],
                                    (T - 1) * C + h * HC :
                                    (T - 1) * C + (h + 1) * HC,
                                ],
                                o_l,
                            )

    _split_multi_waits(nc)
    return nc


def _shard(x: np.ndarray, c: int) -> np.ndarray:
    """Core c's shard, t-plane-major: [ROWS, C, T] -> [ROWS, T, C] -> flat."""
    s = x[c * B_PER_CORE : (c + 1) * B_PER_CORE].reshape(ROWS, C, T)
    return np.ascontiguousarray(s.transpose(0, 2, 1)).reshape(ROWS, FREE)


def _unshard(y: np.ndarray) -> np.ndarray:
    """Invert _shard's layout for one core's int8 0/1 output -> f32."""
    o = (y > 0).astype(np.float32)
    s = o.reshape(ROWS, T, C).transpose(0, 2, 1)
    return np.ascontiguousarray(s).reshape(B_PER_CORE, *FULL_SHAPE[1:])


def kernel(x: np.ndarray) -> np.ndarray:
    assert x.shape == FULL_SHAPE, x.shape
    in_dtype = x.dtype

    if "nc" not in _cache:
        _cache["nc"] = _build_bass()
    nc = _cache["nc"]

    x = np.ascontiguousarray(x, dtype=np.float32)
    in_maps = [{"x": _shard(x, c)} for c in range(N_CORES)]
    res = run_bass_kernel_spmd(nc, in_maps, core_ids=list(range(N_CORES)))
    out = np.concatenate(
        [_unshard(res.results[c]["y"]) for c in range(N_CORES)], axis=0
    )
    return out.astype(in_dtype, copy=False)


# revision 47
# speedup vs baseline: 1.0047x; 1.0047x over previous
"""LIF spiking-neuron recurrence kernel for Trainium2 (Bass/Tile, 8-core SPMD).

Problem: x [32, 128, 32, 32, 8] f32, time on the LAST axis (T=8).
    u_0 = x_0;  o_t = (u_t > Vth);  u_{t+1} = TAU * u_t * (1 - o_t) + x_{t+1}
Output: spikes o [32, 128, 32, 32, 8] f32 (0.0 / 1.0).

Sharding: pure data-parallel over the batch dim (32 -> 4 per core, 8 cores),
no communication. The host lays each core's shard out t-plane-major
([pixels, T] -> [T, pixels] per 1024-pixel row group) so every on-chip
operand is dense unit-stride. Spikes are exactly 0/1 so the output is
stored as int8, cutting store traffic 4x (per-core HBM traffic 21 MB,
~59 us floor at the ~358 GB/s per-core peak).

With cheap stores, the binding constraint is the Vector engine: fp32
tensor_tensor runs at 1 elem/cycle/lane, and the recurrence nominally needs
three 2-tensor ops per timestep (mask, masked-mult, add) = ~5.8 us per
[128, 2048] plane. This kernel reduces DVE to TWO ops per step by computing
the spike gate on the otherwise-idle Activation engine as a Relu ramp:

    z   = Relu(-BIG*u + BIG*u*)        ACT; u* = nextafter(Vth) so that
                                       z > 0  <=>  u <= Vth (exact for every
                                       f32 input; z >= ~12 whenever nonzero)
    w   = min(TAU*u, z)                DVE scalar_tensor_tensor (mult, min)
                                       == TAU*u*(u <= Vth) exactly, because
                                       TAU*u <= 0.0500000008 < 12 when gated
    o_t = Relu(1 - z) -> int8          ACT; z==0 -> 1, z>=12 -> 0
    u'  = w + x_{t+1}                  DVE tensor_tensor add

TAU*u rounds identically to the reference's TAU*u*(1-o) path, so spike
outputs are bit-exact (including u == Vth exactly, handled by u*).

Engine budget per core (measured): DVE 14 stt + 14 tt + final compares
~= 67 us busy; ACT 28 activations + 14 store issues ~= 68 us; DMA ~54 us.
Stores issue from ACT's HWDGE queue right behind the o they store (SWDGE
via GPSIMD measured ~1 us slower end-to-end); loads issue from SP and
prefetch three steps ahead so DMA pressure stays even.

Rejected alternatives (all measured slower):
  - GPSIMD elementwise (tensor_tensor ~3.4 cyc/elem, tensor_scalar
    ~32 us/plane, and it starves DVE via the shared SBUF port)
  - SWDGE accumulate-DMA for the add (~155 GB/s effective and ~2.5-3.5 us
    latency lands on the recurrence critical path)
  - PE identity-matmul adds (fp32 moving operand decomposes to bf16 pairs
    -> not exact), fp16/bf16 state (too many threshold flips for 2e-2)
"""

import numpy as np

import bass_rust
import concourse.bass as bass
import concourse.mybir as mybir
import concourse.tile as tile
from concourse.bass_utils import run_bass_kernel_spmd

VTH = 0.2
TAU = 0.25

# Gate constants: fma(-BIG, u, B_GATE) with B_GATE = float32(BIG * nextafter
# (float32(0.2))). The result is > 0 exactly when u <= float32(0.2) and the
# smallest positive value it takes is ~12 (one f32 ulp at 0.2 scaled by BIG),
# safely above max(TAU*u) = 0.0500000008, so min(TAU*u, z) never picks z when
# the gate is on. Exact under both fused and round-between multiply-add.
BIG = 1.0e9
B_GATE = 200000016.0

N_CORES = 8
FULL_SHAPE = (32, 128, 32, 32, 8)
B_PER_CORE = FULL_SHAPE[0] // N_CORES  # 4
T = FULL_SHAPE[-1]  # 8

ROWS = 256  # per-core partition rows: 4*128*32*32*8 / FREE
FREE = 16384  # free dim per row
C = FREE // T  # 2048 pixels per partition row
N_TILES = ROWS // 128  # 2

_cache: dict = {}


def _split_multi_waits(nc: bass.Bass) -> int:
    """Hoist all-but-one embedded sync waits onto standalone EventSemaphore
    instructions. The walrus build behind bass2jax rejects >1 sync wait per
    instruction ("Too many sync wait commands"); a standalone wait on the
    same engine stream immediately before is semantically identical."""
    n = 0
    for fn in nc.m.functions:
        for block in fn.blocks:
            out = []
            changed = False
            for ins in block.instructions:
                si = ins.sync_info
                waits = list(si.on_wait) if si is not None else []
                if len(waits) > 1:
                    for k, w in enumerate(waits[:-1]):
                        ev = mybir.InstEventSemaphore(
                            name=f"{ins.name}-hw{k}", ins=[], outs=[]
                        )
                        ev.sync_info = bass_rust.SyncInfo(
                            on_wait=[w], on_update=[]
                        )
                        ev.engine = ins.engine
                        nc.inst_map[ev.name] = ev
                        out.append(ev)
                        n += 1
                    si.on_wait = [waits[-1]]
                    changed = True
                out.append(ins)
            if changed:
                block.instructions = out
    return n


def _build_bass() -> bass.Bass:
    f32 = mybir.dt.float32
    i8 = mybir.dt.int8
    Alu = mybir.AluOpType
    Act = mybir.ActivationFunctionType

    nc = bass.Bass(trn_type="TRN2")
    x_d = nc.dram_tensor("x", [ROWS, FREE], f32, kind="ExternalInput")
    y_d = nc.dram_tensor("y", [ROWS, FREE], i8, kind="ExternalOutput")

    # Non-Copy activations take their bias as a per-partition const AP; only
    # 0.0/1.0 are pre-registered, so add the gate bias. Initializing it on
    # the Scalar queue itself (zero, then Copy-with-bias) keeps the init
    # in-order ahead of the first Relu without an all-engine barrier that
    # would hold up the first loads.
    bias_t = nc.alloc_sbuf_tensor("const-bgate", [128, 1], f32)
    nc.scalar.memzero(bias_t.ap())
    nc.scalar.activation(
        bias_t.ap(), bias_t.ap(), mybir.ActivationFunctionType.Copy,
        bias=B_GATE, scale=1.0,
    )
    nc.const_aps.aps[(f32, B_GATE)] = bias_t.ap()

    with tile.TileContext(nc) as tc:
        with (
            tc.tile_pool(name="pin", bufs=12) as pin,
            tc.tile_pool(name="pout", bufs=4) as pout,
            tc.tile_pool(name="pz", bufs=6) as pz,
            tc.tile_pool(name="pu", bufs=4) as pu,
        ):
            row_sl = [slice(i * 128, (i + 1) * 128) for i in range(N_TILES)]

            def load(i, t):
                p = pin.tile([128, C], f32, tag="xp")
                nc.sync.dma_start(p, x_d[row_sl[i], t * C : (t + 1) * C])
                return p

            # three-step prefetch: planes 0-2 of both tiles land first, the
            # rest stream in two steps ahead of their consuming add
            PF = 3
            xp = [[None] * T for _ in range(N_TILES)]
            for t in range(PF):
                for i in range(N_TILES):
                    xp[i][t] = load(i, t)

            u = [xp[i][0] for i in range(N_TILES)]
            for t in range(T - 1):
                # Both tiles' gates are emitted FIRST on ACT each step:
                # z_B must be ready ~4.6us after z_A (when DVE finishes
                # tile A and turns to stt_B), and o/store issues queued in
                # between would push it past that deadline.
                zs = []
                for i in range(N_TILES):
                    if t + PF < T:
                        xp[i][t + PF] = load(i, t + PF)
                    # ACT: spike gate z = Relu(BIG*(u* - u))
                    z = pz.tile([128, C], f32, tag="z")
                    nc.scalar.activation(
                        z, u[i], Act.Relu, bias=B_GATE, scale=-BIG
                    )
                    zs.append(z)
                for i in range(N_TILES):
                    z = zs[i]
                    # ACT: o = Relu(1 - z) in {0, 1} -> int8
                    o_t = pout.tile([128, C], i8, tag="o")
                    nc.scalar.activation(
                        o_t, z, Act.Relu, bias=1.0, scale=-1.0
                    )
                    # DVE: w = min(TAU*u, z) == TAU*u*(u <= Vth), written
                    # straight into the next-state tile, then the add runs
                    # in place - no separate w ring, one fewer allocation
                    # wait per step on the DVE queue
                    un = pu.tile([128, C], f32, tag="u")
                    nc.vector.scalar_tensor_tensor(
                        un, u[i], TAU, z, Alu.mult, Alu.min
                    )
                    # DVE: u' = w + x_{t+1}
                    nc.vector.tensor_tensor(un, un, xp[i][t + 1], Alu.add)
                    # store via ACT's HWDGE queue, right behind the o it
                    # stores (no wait); SWDGE (gpsimd) descriptor-gen runs
                    # in SBUF rings that DVE 2-port ops can lock out
                    nc.scalar.dma_start(
                        y_d[row_sl[i], t * C : (t + 1) * C], o_t
                    )
                    u[i] = un

                    if t == T - 2:
                        # This tile's last-step compare runs right behind
                        # its final add, in halves whose stores overlap, so
                        # the kernel tail is half an op. It stays on DVE:
                        # GPSIMD's tensor_scalar measures ~32us/plane (25x
                        # DVE) and starves DVE via the shared SBUF port.
                        HC = C // 2
                        for h in range(2):
                            o_l = pout.tile([128, HC], i8, tag="oh")
                            nc.vector.tensor_scalar(
                                o_l, un[:, h * HC : (h + 1) * HC],
                                VTH, None, Alu.is_gt,
                            )
                            nc.sync.dma_start(
                                y_d[
                                    row_sl[i],
                                    (T - 1) * C + h * HC :
                                    (T - 1) * C + (h + 1) * HC,
                                ],
                                o_l,
                            )

    _split_multi_waits(nc)
    return nc


def _shard(x: np.ndarray, c: int) -> np.ndarray:
    """Core c's shard, t-plane-major: [ROWS, C, T] -> [ROWS, T, C] -> flat."""
    s = x[c * B_PER_CORE : (c + 1) * B_PER_CORE].reshape(ROWS, C, T)
    return np.ascontiguousarray(s.transpose(0, 2, 1)).reshape(ROWS, FREE)


def _unshard(y: np.ndarray) -> np.ndarray:
    """Invert _shard's layout for one core's int8 0/1 output -> f32."""
    o = (y > 0).astype(np.float32)
    s = o.reshape(ROWS, T, C).transpose(0, 2, 1)
    return np.ascontiguousarray(s).reshape(B_PER_CORE, *FULL_SHAPE[1:])


def kernel(x: np.ndarray) -> np.ndarray:
    assert x.shape == FULL_SHAPE, x.shape
    in_dtype = x.dtype

    if "nc" not in _cache:
        _cache["nc"] = _build_bass()
    nc = _cache["nc"]

    x = np.ascontiguousarray(x, dtype=np.float32)
    in_maps = [{"x": _shard(x, c)} for c in range(N_CORES)]
    res = run_bass_kernel_spmd(nc, in_maps, core_ids=list(range(N_CORES)))
    out = np.concatenate(
        [_unshard(res.results[c]["y"]) for c in range(N_CORES)], axis=0
    )
    return out.astype(in_dtype, copy=False)
